# revision 47
# baseline (speedup 1.0000x reference)
"""Trainium2 Bass kernel for nn_AdaptiveAlphaQuantizedLinear.

out[b,t,k] = sum_n x[b,t,n]*mu1[n] * ((W_q[k,n]-zeros[k,g(n)])*scales[k,g(n)])*mu2[k]
             + bias[k]

Strategy (8 NeuronCores, tensor-parallel along K):
  host prep (layout only on the big tensor):
    - W_q transposed to [N, K] and sharded along K  (int32, full 256MB read on-device)
    - a[k,g] = scales*mu2, c[k,g] = -zeros*scales*mu2 folded host-side (metadata)
    - x' = x*mu1 transposed to [N, BT] bf16; group sums Xg and a ones row are
      appended as 65 extra contraction rows so zeros+bias ride the same matmul
  device per core (one quant group g == one 128-row n-tile):
    - HWDGE DMA streams WqT int32 tiles (1MB, two groups at a time)
    - PE replicates the group scale rows a[g,:] across all 128 partitions via
      ones-outer-product matmuls (4 concurrent 32-row strips, tile_position),
      ACT/DVE copy PSUM -> SBUF (bf16 replicated scale tile)
    - DVE multiplies int32 codes by the replicated scale (mixed-dtype
      tensor_tensor, bf16 out) -- no separate cast needed
    - PE accumulates out[bt, k] = x'T.T @ (a*Wq)T over 64 n-tiles; the
      Xg/ones extra rows close the accumulation with the zeros/bias term
    - ACT copies PSUM -> SBUF, DMA out [256, 1024] f32
  host: concat k-shards, reshape to [8, 32, 8192].
"""
import sys
sys.path.insert(0, "/opt/trn_rl_repo")
import numpy as np

K = 8192
N = 8192
GROUP_SIZE = 128
NG = N // GROUP_SIZE          # 64 groups == 64 n-tiles
B, T = 8, 32
BT = B * T                    # 256
NCORES = 8
KSH = K // NCORES             # 1024 out-features per core
NT = N // 128                 # 64 n-tiles

_NC_CACHE = None


def _build():
    from concourse import bacc, tile, mybir

    bf16 = mybir.dt.bfloat16
    nc = bacc.Bacc("TRN2", target_bir_lowering=False, debug=False,
                   num_devices=NCORES)
    wqt = nc.dram_tensor("wqt", [N, KSH], mybir.dt.int32, kind="ExternalInput")
    xt = nc.dram_tensor("xt", [N, BT], bf16, kind="ExternalInput")
    xgt = nc.dram_tensor("xgt", [NG + 1, BT], bf16, kind="ExternalInput")
    at4 = nc.dram_tensor("at4", [4, NG, KSH], bf16, kind="ExternalInput")
    ct = nc.dram_tensor("ct", [NG + 1, KSH], bf16, kind="ExternalInput")
    out = nc.dram_tensor("out", [BT, KSH], mybir.dt.float32, kind="ExternalOutput")

    with tile.TileContext(nc) as tc:
        with (
            tc.tile_pool(name="const", bufs=1) as cpool,
            tc.tile_pool(name="arow", bufs=6) as arpool,
            tc.tile_pool(name="abc", bufs=6) as abcpool,
            tc.tile_pool(name="wi", bufs=16) as wipool,
            tc.tile_pool(name="ws", bufs=6) as wspool,
            tc.tile_pool(name="psum", bufs=1, space="PSUM") as psum,
            tc.tile_pool(name="psab", bufs=4, space="PSUM") as psab,
            tc.tile_pool(name="outp", bufs=1) as opool,
        ):
            xt_sb = cpool.tile([128, NT, BT], bf16, tag="xt")
            xg_sb = cpool.tile([NG + 1, BT], bf16, tag="xg")
            ct_sb = cpool.tile([NG + 1, KSH], bf16, tag="ct")
            ones_sb = cpool.tile([128, 128], bf16, tag="ones")

            XCH = 8

            def load_xt_chunk(xc):
                tl = NT // XCH
                nc.sync.dma_start(
                    xt_sb[:, xc * tl:(xc + 1) * tl, :],
                    xt[xc * tl * 128:(xc + 1) * tl * 128, :]
                    .rearrange("(t p) d -> p t d", p=128))

            load_xt_chunk(0)
            nc.vector.memset(ones_sb[:], 1.0)

            accs = [psum.tile([128, 512], mybir.dt.float32, tag=f"acc{b}{c}",
                              name=f"acc{b}{c}")
                    for b in range(2) for c in range(2)]

            PAIRS = NT // 2
            LOOKAHEAD = 5
            abcs = {}

            def produce_abc(p):
                # a rows for pair p staged at partitions 0/32/64/96 so the 4
                # outer-product MMs run concurrently in distinct 32-row strips
                a_row = arpool.tile([128, 2 * KSH], bf16, tag="arow",
                                    name="a_row")
                ar_v = a_row[:].rearrange("(h s) k -> h s k", s=32)
                nc.sync.dma_start(
                    ar_v[:, 0, :],
                    at4[:, p * 2:(p + 1) * 2, :])
                a_bc = abcpool.tile([128, 2 * KSH], bf16, tag="abc",
                                    name="a_bc")
                for h in range(4):
                    pab = psab.tile([128, 512], mybir.dt.float32, tag="pab",
                                    name="pab")
                    nc.tensor.matmul(
                        pab[:], ones_sb[32 * h:32 * h + 1, :],
                        a_row[32 * h:32 * h + 1, h * 512:(h + 1) * 512],
                        start=True, stop=True,
                        tile_position=(32 * h, 0),
                    )
                    nc.scalar.copy(a_bc[:, h * 512:(h + 1) * 512], pab[:])
                abcs[p] = a_bc

            for p in range(LOOKAHEAD):
                produce_abc(p)

            for t2 in range(PAIRS):
                # one fully-contiguous 512KB int32 transfer per group (HWDGE)
                wis = []
                for tt in range(2):
                    t = t2 * 2 + tt
                    wig = wipool.tile([128, KSH], mybir.dt.int32, tag="wi",
                                      name="wig")
                    nc.sync.dma_start(wig[:], wqt[t * 128:(t + 1) * 128, :])
                    wis.append(wig)
                if t2 + LOOKAHEAD < PAIRS:
                    produce_abc(t2 + LOOKAHEAD)
                if t2 % 4 == 0 and 1 + t2 // 4 < XCH:
                    load_xt_chunk(1 + t2 // 4)
                if t2 == 24:
                    nc.sync.dma_start(xg_sb[:], xgt[:])
                    nc.sync.dma_start(ct_sb[:], ct[:])
                a_bc = abcs.pop(t2)
                ws = wspool.tile([128, 2, KSH], bf16, tag="ws", name="ws")
                for tt in range(2):
                    t = t2 * 2 + tt
                    nc.vector.tensor_mul(
                        ws[:, tt, :], wis[tt][:],
                        a_bc[:, tt * KSH:(tt + 1) * KSH])
                    for b in range(2):
                        for c in range(2):
                            nc.tensor.matmul(
                                accs[b * 2 + c][:],
                                xt_sb[:, t, b * 128:(b + 1) * 128],
                                ws[:, tt, c * 512:(c + 1) * 512],
                                start=(t == 0), stop=False,
                            )

            # c-term + bias: out[bt,k] += Xg2[bt,g] @ cT[g,k]; closes accumulation
            for b in range(2):
                for c in range(2):
                    nc.tensor.matmul(
                        accs[b * 2 + c][:],
                        xg_sb[:, b * 128:(b + 1) * 128],
                        ct_sb[:, c * 512:(c + 1) * 512],
                        start=False, stop=True,
                    )

            out_sb = opool.tile([128, 2, KSH], mybir.dt.float32, tag="o")
            out_v = out.ap().rearrange("(b p) k -> p b k", p=128)
            for b in range(2):
                for c in range(2):
                    nc.scalar.copy(out_sb[:, b, c * 512:(c + 1) * 512],
                                   accs[b * 2 + c][:])
                    nc.sync.dma_start(
                        out_v[:, b, c * 512:(c + 1) * 512],
                        out_sb[:, b, c * 512:(c + 1) * 512])

    nc.compile()
    return nc


def _get_nc():
    global _NC_CACHE
    if _NC_CACHE is None:
        _NC_CACHE = _build()
    return _NC_CACHE


def _prep_in_maps(x, W_q, scales, zeros, mu1, mu2, bias):
    import ml_dtypes
    bf16 = ml_dtypes.bfloat16
    x2 = np.asarray(x, dtype=np.float32).reshape(BT, N)
    mu1 = np.asarray(mu1, dtype=np.float32)
    mu2 = np.asarray(mu2, dtype=np.float32)
    bias = np.asarray(bias, dtype=np.float32)
    sc = np.asarray(scales, dtype=np.float32)[:, :, 0]   # [K, NG]
    zr = np.asarray(zeros, dtype=np.float32)[:, :, 0]    # [K, NG]
    W_q = np.asarray(W_q)
    assert W_q.dtype == np.int32

    xp = x2 * mu1[None, :]                                # x' [BT, N]
    xt_h = np.ascontiguousarray(xp.T).astype(bf16)        # [N, BT]
    Xg = xp.reshape(BT, NG, GROUP_SIZE).sum(axis=2)       # [BT, NG]
    xgt_h = np.concatenate(
        [np.ascontiguousarray(Xg.T), np.ones((1, BT), np.float32)],
        axis=0).astype(bf16)                              # [NG+1, BT]

    a = sc * mu2[:, None]                                 # [K, NG]
    cmat = -zr * a                                        # [K, NG]

    in_maps = []
    for i in range(NCORES):
        ksl = slice(i * KSH, (i + 1) * KSH)
        wqt_h = np.ascontiguousarray(W_q[ksl, :].T)       # [N, KSH] int32
        at_h = np.ascontiguousarray(a[ksl, :].T).astype(bf16)     # [NG, KSH]
        at4_h = np.ascontiguousarray(np.broadcast_to(at_h, (4, NG, KSH)))
        ct_h = np.concatenate(
            [np.ascontiguousarray(cmat[ksl, :].T),
             bias[None, ksl]], axis=0).astype(bf16)               # [NG+1, KSH]
        in_maps.append({"wqt": wqt_h, "xt": xt_h, "xgt": xgt_h,
                        "at4": at4_h, "ct": ct_h})
    return in_maps


def _run(inputs, trace=False):
    from concourse import bass_utils
    nc = _get_nc()
    in_maps = _prep_in_maps(**inputs)
    res = bass_utils.run_bass_kernel_spmd(
        nc, in_maps, core_ids=list(range(NCORES)), trace=trace)
    out = np.concatenate([res.results[i]["out"] for i in range(NCORES)],
                         axis=1)                          # [BT, K]
    return out.reshape(B, T, K).astype(np.float32), res


def kernel(**inputs) -> np.ndarray:
    out, _ = _run(inputs, trace=False)
    return out


def kernel_traced(**inputs):
    out, res = _run(inputs, trace=True)
    return out, res


# revision 48
# speedup vs baseline: 1.0467x; 1.0467x over previous
"""Trainium2 Bass kernel for nn_AdaptiveAlphaQuantizedLinear.

out[b,t,k] = sum_n x[b,t,n]*mu1[n] * ((W_q[k,n]-zeros[k,g(n)])*scales[k,g(n)])*mu2[k]
             + bias[k]

Strategy (8 NeuronCores, tensor-parallel along K):
  host prep (layout only on the big tensor):
    - W_q transposed to [N, K] and sharded along K  (int32, full 256MB read on-device)
    - a[k,g] = scales*mu2, c[k,g] = -zeros*scales*mu2 folded host-side (metadata)
    - x' = x*mu1 transposed to [N, BT] bf16; group sums Xg and a ones row are
      appended as 65 extra contraction rows so zeros+bias ride the same matmul
  device per core (one quant group g == one 128-row n-tile):
    - HWDGE DMA streams WqT int32 tiles (1MB, two groups at a time)
    - PE replicates the group scale rows a[g,:] across all 128 partitions via
      ones-outer-product matmuls (4 concurrent 32-row strips, tile_position),
      ACT/DVE copy PSUM -> SBUF (bf16 replicated scale tile)
    - DVE multiplies int32 codes by the replicated scale (mixed-dtype
      tensor_tensor, bf16 out) -- no separate cast needed
    - PE accumulates out[bt, k] = x'T.T @ (a*Wq)T over 64 n-tiles; the
      Xg/ones extra rows close the accumulation with the zeros/bias term
    - ACT copies PSUM -> SBUF, DMA out [256, 1024] f32
  host: concat k-shards, reshape to [8, 32, 8192].
"""
import sys
sys.path.insert(0, "/opt/trn_rl_repo")
import numpy as np

K = 8192
N = 8192
GROUP_SIZE = 128
NG = N // GROUP_SIZE          # 64 groups == 64 n-tiles
B, T = 8, 32
BT = B * T                    # 256
NCORES = 8
KSH = K // NCORES             # 1024 out-features per core
NT = N // 128                 # 64 n-tiles

_NC_CACHE = None


def _build():
    from concourse import bacc, tile, mybir

    bf16 = mybir.dt.bfloat16
    nc = bacc.Bacc("TRN2", target_bir_lowering=False, debug=False,
                   num_devices=NCORES)
    wqt = nc.dram_tensor("wqt", [N, KSH], mybir.dt.int32, kind="ExternalInput")
    xt = nc.dram_tensor("xt", [N, BT], bf16, kind="ExternalInput")
    xgt = nc.dram_tensor("xgt", [NG + 1, BT], bf16, kind="ExternalInput")
    at4 = nc.dram_tensor("at4", [4, NG, KSH], bf16, kind="ExternalInput")
    ct = nc.dram_tensor("ct", [NG + 1, KSH], bf16, kind="ExternalInput")
    out = nc.dram_tensor("out", [BT, KSH], mybir.dt.float32, kind="ExternalOutput")

    with tile.TileContext(nc) as tc:
        with (
            tc.tile_pool(name="const", bufs=1) as cpool,
            tc.tile_pool(name="arow", bufs=6) as arpool,
            tc.tile_pool(name="abc", bufs=6) as abcpool,
            tc.tile_pool(name="wi", bufs=12) as wipool,
            tc.tile_pool(name="ws", bufs=6) as wspool,
            tc.tile_pool(name="psum", bufs=1, space="PSUM") as psum,
            tc.tile_pool(name="psab", bufs=4, space="PSUM") as psab,
            tc.tile_pool(name="outp", bufs=1) as opool,
        ):
            xt_sb = cpool.tile([128, NT, BT], bf16, tag="xt")
            xg_sb = cpool.tile([NG + 1, BT], bf16, tag="xg")
            ct_sb = cpool.tile([NG + 1, KSH], bf16, tag="ct")
            ones_sb = cpool.tile([128, 128], bf16, tag="ones")

            XCH = 16

            def load_xt_chunk(xc):
                tl = NT // XCH
                nc.sync.dma_start(
                    xt_sb[:, xc * tl:(xc + 1) * tl, :],
                    xt[xc * tl * 128:(xc + 1) * tl * 128, :]
                    .rearrange("(t p) d -> p t d", p=128))

            load_xt_chunk(0)
            nc.vector.memset(ones_sb[:], 1.0)

            accs = [psum.tile([128, 512], mybir.dt.float32, tag=f"acc{b}{c}",
                              name=f"acc{b}{c}")
                    for b in range(2) for c in range(2)]

            PAIRS = NT // 2
            LOOKAHEAD = 5
            abcs = {}

            def produce_abc(p):
                # a rows for pair p staged at partitions 0/32/64/96 so the 4
                # outer-product MMs run concurrently in distinct 32-row strips
                a_row = arpool.tile([128, 2 * KSH], bf16, tag="arow",
                                    name="a_row")
                ar_v = a_row[:].rearrange("(h s) k -> h s k", s=32)
                nc.sync.dma_start(
                    ar_v[:, 0, :],
                    at4[:, p * 2:(p + 1) * 2, :])
                a_bc = abcpool.tile([128, 2 * KSH], bf16, tag="abc",
                                    name="a_bc")
                for h in range(4):
                    pab = psab.tile([128, 512], mybir.dt.float32, tag="pab",
                                    name="pab")
                    nc.tensor.matmul(
                        pab[:], ones_sb[32 * h:32 * h + 1, :],
                        a_row[32 * h:32 * h + 1, h * 512:(h + 1) * 512],
                        start=True, stop=True,
                        tile_position=(32 * h, 0),
                    )
                    nc.scalar.copy(a_bc[:, h * 512:(h + 1) * 512], pab[:])
                abcs[p] = a_bc

            for p in range(LOOKAHEAD):
                produce_abc(p)

            for t2 in range(PAIRS):
                # one fully-contiguous 512KB int32 transfer per group (HWDGE)
                wis = []
                for tt in range(2):
                    t = t2 * 2 + tt
                    wig = wipool.tile([128, KSH], mybir.dt.int32, tag="wi",
                                      name="wig")
                    nc.sync.dma_start(wig[:], wqt[t * 128:(t + 1) * 128, :])
                    wis.append(wig)
                if t2 + LOOKAHEAD < PAIRS:
                    produce_abc(t2 + LOOKAHEAD)
                if t2 % 2 == 0 and 1 + t2 // 2 < XCH:
                    load_xt_chunk(1 + t2 // 2)
                if t2 == 24:
                    nc.sync.dma_start(xg_sb[:], xgt[:])
                    nc.sync.dma_start(ct_sb[:], ct[:])
                a_bc = abcs.pop(t2)
                ws = wspool.tile([128, 2, KSH], bf16, tag="ws", name="ws")
                for tt in range(2):
                    t = t2 * 2 + tt
                    nc.vector.tensor_mul(
                        ws[:, tt, :], wis[tt][:],
                        a_bc[:, tt * KSH:(tt + 1) * KSH])
                    for b in range(2):
                        for c in range(2):
                            nc.tensor.matmul(
                                accs[b * 2 + c][:],
                                xt_sb[:, t, b * 128:(b + 1) * 128],
                                ws[:, tt, c * 512:(c + 1) * 512],
                                start=(t == 0), stop=False,
                            )

            # c-term + bias: out[bt,k] += Xg2[bt,g] @ cT[g,k]; closes accumulation
            for b in range(2):
                for c in range(2):
                    nc.tensor.matmul(
                        accs[b * 2 + c][:],
                        xg_sb[:, b * 128:(b + 1) * 128],
                        ct_sb[:, c * 512:(c + 1) * 512],
                        start=False, stop=True,
                    )

            out_sb = opool.tile([128, 2, KSH], mybir.dt.float32, tag="o")
            out_v = out.ap().rearrange("(b p) k -> p b k", p=128)
            for b in range(2):
                for c in range(2):
                    nc.scalar.copy(out_sb[:, b, c * 512:(c + 1) * 512],
                                   accs[b * 2 + c][:])
                    nc.sync.dma_start(
                        out_v[:, b, c * 512:(c + 1) * 512],
                        out_sb[:, b, c * 512:(c + 1) * 512])

    nc.compile()
    return nc


def _get_nc():
    global _NC_CACHE
    if _NC_CACHE is None:
        _NC_CACHE = _build()
    return _NC_CACHE


def _prep_in_maps(x, W_q, scales, zeros, mu1, mu2, bias):
    import ml_dtypes
    bf16 = ml_dtypes.bfloat16
    x2 = np.asarray(x, dtype=np.float32).reshape(BT, N)
    mu1 = np.asarray(mu1, dtype=np.float32)
    mu2 = np.asarray(mu2, dtype=np.float32)
    bias = np.asarray(bias, dtype=np.float32)
    sc = np.asarray(scales, dtype=np.float32)[:, :, 0]   # [K, NG]
    zr = np.asarray(zeros, dtype=np.float32)[:, :, 0]    # [K, NG]
    W_q = np.asarray(W_q)
    assert W_q.dtype == np.int32

    xp = x2 * mu1[None, :]                                # x' [BT, N]
    xt_h = np.ascontiguousarray(xp.T).astype(bf16)        # [N, BT]
    Xg = xp.reshape(BT, NG, GROUP_SIZE).sum(axis=2)       # [BT, NG]
    xgt_h = np.concatenate(
        [np.ascontiguousarray(Xg.T), np.ones((1, BT), np.float32)],
        axis=0).astype(bf16)                              # [NG+1, BT]

    a = sc * mu2[:, None]                                 # [K, NG]
    cmat = -zr * a                                        # [K, NG]

    in_maps = []
    for i in range(NCORES):
        ksl = slice(i * KSH, (i + 1) * KSH)
        wqt_h = np.ascontiguousarray(W_q[ksl, :].T)       # [N, KSH] int32
        at_h = np.ascontiguousarray(a[ksl, :].T).astype(bf16)     # [NG, KSH]
        at4_h = np.ascontiguousarray(np.broadcast_to(at_h, (4, NG, KSH)))
        ct_h = np.concatenate(
            [np.ascontiguousarray(cmat[ksl, :].T),
             bias[None, ksl]], axis=0).astype(bf16)               # [NG+1, KSH]
        in_maps.append({"wqt": wqt_h, "xt": xt_h, "xgt": xgt_h,
                        "at4": at4_h, "ct": ct_h})
    return in_maps


def _run(inputs, trace=False):
    from concourse import bass_utils
    nc = _get_nc()
    in_maps = _prep_in_maps(**inputs)
    res = bass_utils.run_bass_kernel_spmd(
        nc, in_maps, core_ids=list(range(NCORES)), trace=trace)
    out = np.concatenate([res.results[i]["out"] for i in range(NCORES)],
                         axis=1)                          # [BT, K]
    return out.reshape(B, T, K).astype(np.float32), res


def kernel(**inputs) -> np.ndarray:
    out, _ = _run(inputs, trace=False)
    return out


def kernel_traced(**inputs):
    out, res = _run(inputs, trace=True)
    return out, res
